# revision 89
# baseline (speedup 1.0000x reference)
"""Trainium2 Bass kernel for a 1D Kernel Neural Operator (KNO) on a regular grid.

Reference computation (N=2048 nodes, C=32 channels, DEPTH=3):
    fq = gelu([f_x, x] @ lift_W.T + lift_b)
    for i in 0..2:
        skip  = fq @ pw_W[i].T + pw_b[i]
        K_c   = sig2_c * exp(-(x_n - x_q)^2 * a_c),  a_c = 1/(2*ell2_c)
        integ = einsum('cnq,qc->nc', K, fq * w)
        fq    = skip + integ; gelu if i < 2
    out = (gelu(gelu(fq@W1.T+b1)@W2.T+b2)) @ W3.T + b3

Instead of materializing the C x N x N kernels, we use the factorization
exp(-a(x_n-x_q)^2) = e^{-a x_n^2} e^{2 a x_n x_q} e^{-a x_q^2} with the Taylor
expansion e^{2a x_n x_q} = sum_k (2a)^k/k! x_n^k x_q^k (K=32 terms). Each
layer's integral is two matmuls through the moment basis V[n,k] = x_n^k:
    U       = fq ⊙ (w_q e^{-a_c x_q^2})             [N,C]
    M[k,c]  = sum_q V[q,k] U[q,c]                   [K,C]   (PE, 16 psum-accum)
    Mt      = M ⊙ B,  B[k,c] = (2a_c)^k/k!
    integ   = (s2_c e^{-a_c x_n^2}) ⊙ (VT.T @ Mt)   [N,C]   (PE)

All constant tensors (the gelu'd lift of the inputs, V, VT, the Gaussian
envelopes, B, block-diag mixing weights) are pure functions of the inputs and
are precomputed on the host; the device program is the KNO operator layers +
projection head. All matmul operands are bf16; PSUM accumulation stays fp32.

Data layout: the [N,C] state lives channel-transposed in a 4-chunk stack
fqT[32j + c, n'] = fq[512j + n', c], a single [128, 512] SBUF tile. Channel
mixing (pw/proj) is ONE matmul per layer with host-built block-diagonal
[128,128] weights; biases ride the gelu activation's per-partition bias port.
The moment contraction needs q on partitions, so each layer does 4 PE
transposes of fqT back to natural layout (identity built on-device with one
affine_select); per-half transpose tiles keep the dependency tracking exact.
Moment matmuls use a 4x-replicated basis (stride-0 broadcast DVE copies) so
M lands replicated across partition groups and one broadcast multiply against
a block-diag-masked B4 builds the [128,128] eval stationary. The skip PSUM
banks stay open through layers 0/1: the integral joins them via a PE
identity-matmul accumulate and the gelu reads the sum straight from PSUM.
Layer 2 has no gelu, so proj1 distributes over it: pp1 = (proj1 pw2)^T fq1
(PE, early, accumulation group left open) + proj1^T z2 (PE accumulate) --
layer 2's skip matmul, combine-add and the standalone proj1 matmul all
disappear from the chain.

DMA scheduling: the kernel is LATENCY-bound -- each DMA ring slot lands
~2us after the previous one almost regardless of size, so the layer-0
critical tensors are MERGED into the three slot-1 packs:
    sync  slot1: fqT | Vm (half 0)          slot2: VTs    slot3: dl2
    act   slot1: fqT | Vm (half 1) | B4M0   slot2: dl1    slot3: dhd
    gpsimd slot1: tmpN0 | pwW0 | cf (f32 bitcast)  slot2: EsT0
eval/combine/gelu are split into 256-column halves so PE/DVE/ACT pipeline
across halves; the skip matmuls fill PE idle slots during the transposes.

Sharding: the whole problem is a dependent chain of small ops, so all 8 cores
run identical replicas (collectives cost more than they save); the output is
taken from core 0.
"""

import numpy as np
import ml_dtypes

import concourse.bass as bass
import concourse.tile as tile
from concourse import bacc, mybir
from concourse.bass_utils import run_bass_kernel_spmd

N = 2048
C = 32
K = 16   # Taylor terms: fp64 truncation err 6e-5, far below the bf16 floor
DEPTH = 3
NCORES = 8
F32 = mybir.dt.float32
BF16 = mybir.dt.bfloat16
AF = mybir.ActivationFunctionType
ALU = mybir.AluOpType
NPBF16 = ml_dtypes.bfloat16

DA_W = 384     # fqT half | Vm half
DB_W = 512     # fqT half | Vm half | B4M0 (rows 0:64)
DG_W = 650     # tmpN0 + pwW0 + cf (5 x f32 as 10 bf16 cols)
DE0_W = 512    # EsT0
DHD_W = 388    # p1W + p2W + sel3 + (proj1 pw2) block-diag
DL_W = 1280    # per-layer: tmpN + EsT + B4M + pwW  (layers 1, 2)
CF_W = 5
LSPLIT = [(0, 256), (256, 512)]  # layer halves

_CACHE = {}


def build_program(nc):
    da = nc.dram_tensor("da", [128, DA_W], BF16, kind="ExternalInput")
    db = nc.dram_tensor("db", [128, DB_W], BF16, kind="ExternalInput")
    dg = nc.dram_tensor("dg", [128, DG_W], BF16, kind="ExternalInput")
    dy = nc.dram_tensor("dy", [4 * K, 512], BF16, kind="ExternalInput")
    de0 = nc.dram_tensor("de0", [128, DE0_W], BF16, kind="ExternalInput")
    dhd = nc.dram_tensor("dhd", [128, DHD_W], BF16, kind="ExternalInput")
    dl1 = nc.dram_tensor("dl1", [128, DL_W], BF16, kind="ExternalInput")
    dl2 = nc.dram_tensor("dl2", [128, DL_W], BF16, kind="ExternalInput")
    out_dram = nc.dram_tensor("out", [4, 512], F32, kind="ExternalOutput")

    with tile.TileContext(nc) as tc:
        with (
            tc.tile_pool(name="const", bufs=1) as cp,
            tc.tile_pool(name="work", bufs=3) as wp,
            tc.tile_pool(name="pmix", bufs=2, space="PSUM") as pmix,
            tc.tile_pool(name="ptr", bufs=1, space="PSUM") as ptr,
            tc.tile_pool(name="pmom", bufs=1, space="PSUM") as pmom,
            tc.tile_pool(name="pev", bufs=1, space="PSUM") as pev,
            tc.tile_pool(name="pout", bufs=2, space="PSUM") as pop,
        ):
            # warm-act source and home of the eval stationary
            Mt4 = cp.tile([4 * K, 128], BF16, tag="Mt4")
            nc.vector.memset(Mt4[:], 0.0)

            # slot-1 packs: everything layer 0 touches early
            dat = cp.tile([128, DA_W], BF16, tag="dat")
            nc.sync.dma_start(dat[:], da[:])
            dyt = cp.tile([4 * K, 512], BF16, tag="dyt")
            nc.sync.dma_start(dyt[:], dy[:])
            dl2t = cp.tile([128, DL_W], BF16, tag="dl2t")
            nc.sync.dma_start(dl2t[:], dl2[:])

            dbt = cp.tile([128, DB_W], BF16, tag="dbt")
            nc.scalar.dma_start(dbt[:], db[:])

            dgt = cp.tile([128, DG_W], BF16, tag="dgt")
            nc.gpsimd.dma_start(dgt[:], dg[:])
            de0t = cp.tile([128, DE0_W], BF16, tag="de0t")
            nc.gpsimd.dma_start(de0t[:], de0[:])
            # on-device transpose identity: ones tile + diagonal select
            # (issued after both gpsimd DMAs; still ready before the first
            # transpose, which waits on the slot-1 state pack)
            ident = cp.tile([128, 128], BF16, tag="ident")
            nc.gpsimd.memset(ident[:], 1.0)
            nc.gpsimd.affine_select(
                ident[:], ident[:], [[-1, 128]],
                ALU.is_equal, 0.0, base=0, channel_multiplier=1,
            )

            # prefetch the gelu activation table during the DMA wait
            warm = wp.tile([1, 8], BF16, tag="warm")
            nc.scalar.activation(warm[:], Mt4[0:1, 0:8], AF.Gelu_apprx_tanh)

            dl1t = cp.tile([128, DL_W], BF16, tag="dl1t")
            nc.scalar.dma_start(dl1t[:], dl1[:])
            dhdt = cp.tile([128, DHD_W], BF16, tag="dhdt")
            nc.scalar.dma_start(dhdt[:], dhd[:])

            # accessor lists; layer-0 sources live split across the two
            # slot-1 packs (half h in dat/dbt)
            fqh0 = [dat[:, 0:256], dbt[:, 0:256]]
            Vmh = [dat[:, 256:256 + 8 * K], dbt[:, 256:256 + 8 * K]]
            tmpN = [dgt[:, 0:512], dl1t[:, 0:512], dl2t[:, 0:512]]
            EsT = [de0t[:, 0:512], dl1t[:, 512:1024], dl2t[:, 512:1024]]
            VTs = dyt[:, 0:512]
            pwW = [dgt[:, 512:640], dl1t[:, 1152:1280], None]
            B4M = [dbt[0:4 * K, 384:512], dl1t[0:4 * K, 1024:1152],
                   dl2t[0:4 * K, 1024:1152]]
            cfv = dgt[:, 640:650].bitcast(F32)
            pwb = [cfv[:, 0:1], cfv[:, 1:2]]
            p1b = cfv[:, 2:3]
            p2b = cfv[:, 3:4]
            b3c = cfv[0:4, 4:5]
            p1W = dhdt[:, 0:128]
            p2W = dhdt[:, 128:256]
            sel3 = dhdt[:, 256:260]
            WpW = dhdt[:, 260:388]  # (proj1_W @ pw_W[2]) block-diag

            # 4x-replicated moment basis via stride-0 broadcast copies; the
            # Vm halves land with the state, so DVE builds these while the
            # PE transposes run
            Vm4 = cp.tile([128, 64 * K], BF16, tag="Vm4")
            for half in range(2):
                nc.vector.tensor_copy(
                    Vm4[:, 32 * K * half:32 * K * (half + 1)]
                    .rearrange("p (t r k) -> p t r k", t=8, r=4),
                    Vmh[half].rearrange("p (t k) -> p t k", t=8)
                    .unsqueeze(2).broadcast_to((128, 8, 4, K)),
                )

            fq = None  # layer-0 state is read from fqh0

            def fq_block(m):
                if fq is None:
                    return fqh0[m // 2][:, 128 * (m % 2):128 * (m % 2 + 1)]
                return fq[:, 128 * m:128 * (m + 1)]

            def fq_half(h):
                if fq is None:
                    return fqh0[h]
                return fq[:, 256 * h:256 * (h + 1)]

            # ---------------- KNO layers ----------------
            g1 = None
            for i in range(DEPTH):
                last = i == DEPTH - 1
                # natural layout: trp[p, 128m + 32j + c] = fq[512j+128m+p, c]
                # -- one tile per half so the U-multiplies wait only on their
                # own half's transposes (dependency tracking is per-tile)
                trph = [
                    ptr.tile([128, 256], BF16, tag="trpA", name=f"trpA_{i}"),
                    ptr.tile([128, 256], BF16, tag="trpB", name=f"trpB_{i}"),
                ]
                # one PSUM bank per half: the accumulation group (skip /
                # folded-proj1 matmul, later joined by the integral via a PE
                # identity accumulate) stays open over the whole layer
                skph = [
                    pmix.tile([128, 512], F32, tag="mix", name=f"skpA_{i}"),
                    pmix.tile([128, 512], F32, tag="mix", name=f"skpB_{i}"),
                ]
                U = wp.tile([128, 512], BF16, tag="U")
                for m in range(4):
                    nc.tensor.transpose(
                        trph[m // 2][:, 128 * (m % 2):128 * (m % 2 + 1)],
                        fq_block(m),
                        ident[:],
                    )
                # U = fq_nat * (w e^{-a x^2}), halves so moments start early
                for h, (lo, hi) in enumerate(LSPLIT):
                    nc.vector.tensor_mul(
                        U[:, lo:hi], trph[h][:, 0:256], tmpN[i][:, lo:hi]
                    )
                # replicated basis -> M lands on all 4 partition groups
                Mp4 = pmom.tile([4 * K, C], F32, tag="Mp4", name=f"Mp4_{i}")
                for t in range(16):
                    m, j = divmod(t, 4)
                    nc.tensor.matmul(
                        Mp4[:],
                        Vm4[:, 4 * K * t:4 * K * (t + 1)],
                        U[:, 128 * m + 32 * j:128 * m + 32 * j + 32],
                        start=(t == 0),
                        stop=(t == 15),
                    )
                # skip^T via block-diagonal weights in the PE idle slot here;
                # layer 2 folds proj1 in: pp1 = (proj1 pw2)^T fq1 + proj1^T z2
                for h, (lo, hi) in enumerate(LSPLIT):
                    nc.tensor.matmul(
                        skph[h][:, lo:hi], WpW if last else pwW[i],
                        fq_half(h), start=True, stop=False,
                    )
                nc.vector.tensor_mul(
                    Mt4[:].rearrange("p (r c) -> p r c", r=4),
                    Mp4[:].unsqueeze(1).broadcast_to((4 * K, 4, C)),
                    B4M[i].rearrange("p (r c) -> p r c", r=4),
                )
                # eval + combine, pipelined in column halves:
                # fq_next = gelu(skip + pw_b + Es * (Mt4^T @ VTs))
                PT = pev.tile([128, 512], F32, tag="PT")
                z = wp.tile([128, 512], BF16, tag="z")
                fq2 = wp.tile([128, 512], BF16, tag="fq", name=f"fq2_{i}")
                for lo, hi in LSPLIT:
                    nc.tensor.matmul(
                        PT[:, lo:hi], Mt4[:], VTs[:, lo:hi], start=True, stop=True
                    )
                for h, (lo, hi) in enumerate(LSPLIT):
                    sl = slice(lo, hi)
                    nc.vector.tensor_mul(z[:, sl], PT[:, sl], EsT[i][:, sl])
                    # PE accumulate of the integral onto the open skip bank
                    # (identity for layers 0/1; proj1 mixing for layer 2),
                    # then gelu straight from PSUM
                    nc.tensor.matmul(
                        skph[h][:, sl], p1W if last else ident[:], z[:, sl],
                        start=False, stop=True,
                    )
                    nc.scalar.activation(
                        fq2[:, sl], skph[h][:, sl], AF.Gelu_apprx_tanh,
                        bias=p1b if last else pwb[i],
                    )
                fq = fq2
            g1 = fq  # = gelu(proj1 @ (skip2 + integ2) + p1b)

            # ------------- projection head tail (stage-major) ---------------
            # stage-major emission so the PE never runs a later stage of
            # chunk 0 ahead of an earlier stage of chunk 1
            pp2h = [
                pmix.tile([128, 512], F32, tag="mix", name="pp2A"),
                pmix.tile([128, 512], F32, tag="mix", name="pp2B"),
            ]
            g2 = wp.tile([128, 512], BF16, tag="fq")
            pouth = [
                pop.tile([4, 512], F32, tag="pout", name="poutA"),
                pop.tile([4, 512], F32, tag="pout", name="poutB"),
            ]
            outsb = wp.tile([4, 512], F32, tag="outsb")
            for h, (lo, hi) in enumerate(LSPLIT):
                sl = slice(lo, hi)
                nc.tensor.matmul(pp2h[h][:, sl], p2W, g1[:, sl], start=True, stop=True)
                nc.scalar.activation(
                    g2[:, sl], pp2h[h][:, sl], AF.Gelu_apprx_tanh, bias=p2b
                )
            for h, (lo, hi) in enumerate(LSPLIT):
                sl = slice(lo, hi)
                # final dot: proj3_W folded into a selection stationary; the
                # output bias rides the SBUF-staging Identity's bias port on
                # the now-idle ACT engine (Identity shares the gelu table set)
                nc.tensor.matmul(pouth[h][:, sl], sel3, g2[:, sl], start=True, stop=True)
                nc.scalar.activation(
                    outsb[:, sl], pouth[h][:, sl], AF.Identity, bias=b3c
                )
                eng = nc.gpsimd if lo == 0 else nc.sync
                eng.dma_start(out_dram[:, sl], outsb[:, sl])

    return nc


def get_nc():
    if "nc" not in _CACHE:
        nc = bacc.Bacc("TRN2", target_bir_lowering=False, debug=False, num_devices=NCORES)
        build_program(nc)
        nc.compile()
        _CACHE["nc"] = nc
    return _CACHE["nc"]


def make_in_map(
    f_x, x_grid, q_weights, lift_W, lift_b, pw_W, pw_b, ker_log_ell, ker_log_sigma,
    proj1_W, proj1_b, proj2_W, proj2_b, proj3_W, proj3_b,
):
    f8 = lambda a: np.asarray(a, dtype=np.float64)
    x = f8(x_grid).reshape(N)
    w = f8(q_weights).reshape(N)
    f = f8(f_x).reshape(N)
    a = 0.5 * np.exp(-2.0 * f8(ker_log_ell))          # [DEPTH, C]
    sig2 = np.exp(2.0 * f8(ker_log_sigma))            # [DEPTH, C]
    ks = np.arange(K, dtype=np.float64)
    lnfact = np.concatenate([[0.0], np.cumsum(np.log(np.arange(1, K)))])

    p = np.arange(128)
    npr = np.arange(512)

    # host lift: fqT[32j+c, n'] = gelu([f,x] @ lift_W.T + lift_b)[512j+n', c]
    pre = np.stack([f, x], axis=1) @ f8(lift_W).T + f8(lift_b)  # [N, C]
    fql = 0.5 * pre * (1.0 + np.tanh(
        0.7978845608028654 * (pre + 0.044715 * pre ** 3)))
    fqT = np.zeros((128, 512), np.float64)
    for j in range(4):
        fqT[32 * j:32 * (j + 1), :] = fql[512 * j:512 * (j + 1), :].T

    def bd(W):  # block-diag lhsT: [32j+c', 32j+c] = W[c, c']
        M = np.zeros((128, 128), np.float64)
        for j in range(4):
            M[32 * j:32 * (j + 1), 32 * j:32 * (j + 1)] = f8(W).T
        return M

    def tmpN_of(i):
        t = np.zeros((128, 512), np.float64)
        for m in range(4):
            for j in range(4):
                q = 512 * j + 128 * m + p
                t[:, 128 * m + 32 * j:128 * m + 32 * j + 32] = (
                    w[q, None] * np.exp(-a[i][None, :] * (x[q, None] ** 2)))
        return t

    def EsT_of(i):
        e = np.zeros((128, 512), np.float64)
        for j in range(4):
            nn = 512 * j + npr
            e[32 * j:32 * (j + 1), :] = (
                sig2[i][:, None] * np.exp(-a[i][:, None] * (x[None, nn] ** 2)))
        return e

    def B4M_of(i):  # block-diag B[k,c] = (2 a_c)^k / k!  ([4K, 128])
        M = np.zeros((4 * K, 128), np.float64)
        B = np.exp(ks[:, None] * np.log(2.0 * a[i][None, :]) - lnfact[:, None])
        for j in range(4):
            M[K * j:K * (j + 1), 32 * j:32 * (j + 1)] = B
        return M

    # moment basis Vm[p, K(4m+j)+k] = x_{512j+128m+p}^k
    dvm = np.zeros((128, 16 * K), np.float64)
    for m in range(4):
        for j in range(4):
            q = 512 * j + 128 * m + p
            dvm[:, K * (4 * m + j):K * (4 * m + j + 1)] = x[q, None] ** ks[None, :]

    b0 = np.zeros((128, 128), np.float64)
    b0[0:4 * K, :] = B4M_of(0)
    da = np.concatenate([fqT[:, 0:256], dvm[:, 0:8 * K]], axis=1)
    db = np.concatenate(
        [fqT[:, 256:512], dvm[:, 8 * K:16 * K], b0], axis=1)

    cfa = np.zeros((128, CF_W), np.float64)
    cfa[:, 0] = np.tile(f8(pw_b)[0], 4)
    cfa[:, 1] = np.tile(f8(pw_b)[1], 4)
    cfa[:, 2] = np.tile(f8(proj1_b) + f8(proj1_W) @ f8(pw_b)[2], 4)
    cfa[:, 3] = np.tile(f8(proj2_b), 4)
    cfa[0:4, 4] = f8(proj3_b)[0]
    cf16 = cfa.astype(np.float32).view(NPBF16)  # raw f32 halves as bf16 cols

    dgp = np.concatenate(
        [tmpN_of(0).astype(NPBF16), bd(pw_W[0]).astype(NPBF16), cf16], axis=1)

    dy = np.zeros((4 * K, 512), np.float64)
    for j in range(4):
        dy[K * j:K * (j + 1), 0:512] = (
            x[None, 512 * j:512 * (j + 1)] ** ks[:, None])

    dhd = np.zeros((128, DHD_W), np.float64)
    dhd[:, 0:128] = bd(proj1_W)
    dhd[:, 128:256] = bd(proj2_W)
    for j in range(4):
        dhd[32 * j:32 * (j + 1), 256 + j] = f8(proj3_W)[0]
    dhd[:, 260:388] = bd(f8(proj1_W) @ f8(pw_W[2]))

    def dl_of(i):
        dl = np.zeros((128, DL_W), np.float64)
        dl[:, 0:512] = tmpN_of(i)
        dl[:, 512:1024] = EsT_of(i)
        dl[0:4 * K, 1024:1152] = B4M_of(i)
        dl[:, 1152:1280] = bd(pw_W[i])
        return dl

    return {
        "da": da.astype(NPBF16),
        "db": db.astype(NPBF16),
        "dg": dgp,
        "dy": dy.astype(NPBF16),
        "de0": EsT_of(0).astype(NPBF16),
        "dhd": dhd.astype(NPBF16),
        "dl1": dl_of(1).astype(NPBF16),
        "dl2": dl_of(2).astype(NPBF16),
    }


def kernel(**inputs) -> np.ndarray:
    nc = get_nc()
    in_map = make_in_map(**inputs)
    res = run_bass_kernel_spmd(nc, [in_map] * NCORES, list(range(NCORES)))
    return np.asarray(res.results[0]["out"], dtype=np.float32).reshape(N)


# revision 97
# speedup vs baseline: 1.0434x; 1.0434x over previous
"""Trainium2 Bass kernel for a 1D Kernel Neural Operator (KNO) on a regular grid.

Reference computation (N=2048 nodes, C=32 channels, DEPTH=3):
    fq = gelu([f_x, x] @ lift_W.T + lift_b)
    for i in 0..2:
        skip  = fq @ pw_W[i].T + pw_b[i]
        K_c   = sig2_c * exp(-(x_n - x_q)^2 * a_c),  a_c = 1/(2*ell2_c)
        integ = einsum('cnq,qc->nc', K, fq * w)
        fq    = skip + integ; gelu if i < 2
    out = (gelu(gelu(fq@W1.T+b1)@W2.T+b2)) @ W3.T + b3

Instead of materializing the C x N x N kernels, we use the factorization
exp(-a(x_n-x_q)^2) = e^{-a x_n^2} e^{2 a x_n x_q} e^{-a x_q^2} with the Taylor
expansion e^{2a x_n x_q} = sum_k (2a)^k/k! x_n^k x_q^k (K=32 terms). Each
layer's integral is two matmuls through the moment basis V[n,k] = x_n^k:
    U       = fq ⊙ (w_q e^{-a_c x_q^2})             [N,C]
    M[k,c]  = sum_q V[q,k] U[q,c]                   [K,C]   (PE, 16 psum-accum)
    Mt      = M ⊙ B,  B[k,c] = (2a_c)^k/k!
    integ   = (s2_c e^{-a_c x_n^2}) ⊙ (VT.T @ Mt)   [N,C]   (PE)

All constant tensors (the gelu'd lift of the inputs, V, VT, the Gaussian
envelopes, B, block-diag mixing weights) are pure functions of the inputs and
are precomputed on the host; the device program is the KNO operator layers +
projection head. All matmul operands are bf16; PSUM accumulation stays fp32.

Data layout: the [N,C] state lives channel-transposed in a 4-chunk stack
fqT[32j + c, n'] = fq[512j + n', c], a single [128, 512] SBUF tile. Channel
mixing (pw/proj) is ONE matmul per layer with host-built block-diagonal
[128,128] weights; biases ride the gelu activation's per-partition bias port.
The moment contraction needs q on partitions, so each layer does 4 PE
transposes of fqT back to natural layout (identity built on-device with one
affine_select); per-half transpose tiles keep the dependency tracking exact.
Moment matmuls use a 4x-replicated basis (stride-0 broadcast DVE copies) so
M lands replicated across partition groups and one broadcast multiply against
a block-diag-masked B4 builds the [128,128] eval stationary. The skip PSUM
banks stay open through layers 0/1: the integral joins them via a PE
identity-matmul accumulate and the gelu reads the sum straight from PSUM.
Layer 2 has no gelu, so proj1 distributes over it: pp1 = (proj1 pw2)^T fq1
(PE, early, accumulation group left open) + proj1^T z2 (PE accumulate) --
layer 2's skip matmul, combine-add and the standalone proj1 matmul all
disappear from the chain.

DMA scheduling: the kernel is LATENCY-bound -- each DMA ring slot lands
~2us after the previous one almost regardless of size, so the layer-0
critical tensors are MERGED into the three slot-1 packs:
    sync  slot1: fqT | Vm (half 0)          slot2: VTs    slot3: dl2
    act   slot1: fqT | Vm (half 1) | B4M0   slot2: dl1    slot3: dhd
    gpsimd slot1: tmpN0 | pwW0 | cf (f32 bitcast)  slot2: EsT0
eval/combine/gelu are split into 256-column halves so PE/DVE/ACT pipeline
across halves; the skip matmuls fill PE idle slots during the transposes.

Sharding: the whole problem is a dependent chain of small ops, so all 8 cores
run identical replicas (collectives cost more than they save); the output is
taken from core 0.
"""

import numpy as np
import ml_dtypes

import concourse.bass as bass
import concourse.tile as tile
from concourse import bacc, mybir
from concourse.bass_utils import run_bass_kernel_spmd

N = 2048
C = 32
K = 16   # Taylor terms: fp64 truncation err 6e-5, far below the bf16 floor
DEPTH = 3
NCORES = 8
F32 = mybir.dt.float32
BF16 = mybir.dt.bfloat16
AF = mybir.ActivationFunctionType
ALU = mybir.AluOpType
NPBF16 = ml_dtypes.bfloat16

DA_W = 384     # fqT half | Vm half
DB_W = 512     # fqT half | Vm half | B4M0
DG_W = 650     # tmpN0 + pwW0 + cf (5 x f32 as 10 bf16 cols)
DE0_W = 512    # EsT0
DHD_W = 388    # p1W + p2W + sel3 + (proj1 pw2) block-diag
DL_W = 1280    # per-layer: tmpN + EsT + B4M + pwW  (layers 1, 2)
CF_W = 5
LSPLIT = [(0, 256), (256, 512)]  # layer halves

_CACHE = {}


def build_program(nc):
    da = nc.dram_tensor("da", [128, DA_W], BF16, kind="ExternalInput")
    db = nc.dram_tensor("db", [128, DB_W], BF16, kind="ExternalInput")
    dg = nc.dram_tensor("dg", [128, DG_W], BF16, kind="ExternalInput")
    dy = nc.dram_tensor("dy", [128, 512], BF16, kind="ExternalInput")
    de0 = nc.dram_tensor("de0", [128, DE0_W], BF16, kind="ExternalInput")
    dhd = nc.dram_tensor("dhd", [128, DHD_W], BF16, kind="ExternalInput")
    dl1 = nc.dram_tensor("dl1", [128, DL_W], BF16, kind="ExternalInput")
    dl2 = nc.dram_tensor("dl2", [128, DL_W], BF16, kind="ExternalInput")
    out_dram = nc.dram_tensor("out", [4, 512], F32, kind="ExternalOutput")

    with tile.TileContext(nc) as tc:
        with (
            tc.tile_pool(name="const", bufs=1) as cp,
            tc.tile_pool(name="work", bufs=3) as wp,
            tc.tile_pool(name="pmix", bufs=2, space="PSUM") as pmix,
            tc.tile_pool(name="ptr", bufs=1, space="PSUM") as ptr,
            tc.tile_pool(name="pmom", bufs=1, space="PSUM") as pmom,
            tc.tile_pool(name="pev", bufs=1, space="PSUM") as pev,
            tc.tile_pool(name="pout", bufs=2, space="PSUM") as pop,
        ):
            # warm-act source and home of the eval stationary
            Mt4 = cp.tile([128, 128], BF16, tag="Mt4")
            nc.vector.memset(Mt4[:], 0.0)

            # slot-1 packs: everything layer 0 touches early
            dat = cp.tile([128, DA_W], BF16, tag="dat")
            nc.sync.dma_start(dat[:], da[:])
            dyt = cp.tile([128, 512], BF16, tag="dyt")
            nc.sync.dma_start(dyt[:], dy[:])
            dl2t = cp.tile([128, DL_W], BF16, tag="dl2t")
            nc.sync.dma_start(dl2t[:], dl2[:])

            dbt = cp.tile([128, DB_W], BF16, tag="dbt")
            nc.scalar.dma_start(dbt[:], db[:])

            dgt = cp.tile([128, DG_W], BF16, tag="dgt")
            nc.gpsimd.dma_start(dgt[:], dg[:])
            de0t = cp.tile([128, DE0_W], BF16, tag="de0t")
            nc.gpsimd.dma_start(de0t[:], de0[:])
            # on-device transpose identity: ones tile + diagonal select
            # (issued after both gpsimd DMAs; still ready before the first
            # transpose, which waits on the slot-1 state pack)
            ident = cp.tile([128, 128], BF16, tag="ident")
            nc.gpsimd.memset(ident[:], 1.0)
            nc.gpsimd.affine_select(
                ident[:], ident[:], [[-1, 128]],
                ALU.is_equal, 0.0, base=0, channel_multiplier=1,
            )

            # prefetch the gelu activation table during the DMA wait
            warm = wp.tile([1, 8], BF16, tag="warm")
            nc.scalar.activation(warm[:], Mt4[0:1, 0:8], AF.Gelu_apprx_tanh)

            dl1t = cp.tile([128, DL_W], BF16, tag="dl1t")
            nc.scalar.dma_start(dl1t[:], dl1[:])
            dhdt = cp.tile([128, DHD_W], BF16, tag="dhdt")
            nc.scalar.dma_start(dhdt[:], dhd[:])

            # accessor lists; layer-0 sources live split across the two
            # slot-1 packs (half h in dat/dbt)
            fqh0 = [dat[:, 0:256], dbt[:, 0:256]]
            Vmh = [dat[:, 256:256 + 8 * K], dbt[:, 256:256 + 8 * K]]
            tmpN = [dgt[:, 0:512], dl1t[:, 0:512], dl2t[:, 0:512]]
            EsT = [de0t[:, 0:512], dl1t[:, 512:1024], dl2t[:, 512:1024]]
            VTs = dyt[:, 0:512]
            pwW = [dgt[:, 512:640], dl1t[:, 1152:1280], None]
            B4M = [dbt[:, 384:512], dl1t[:, 1024:1152], dl2t[:, 1024:1152]]
            cfv = dgt[:, 640:650].bitcast(F32)
            pwb = [cfv[:, 0:1], cfv[:, 1:2]]
            p1b = cfv[:, 2:3]
            p2b = cfv[:, 3:4]
            b3c = cfv[0:4, 4:5]
            p1W = dhdt[:, 0:128]
            p2W = dhdt[:, 128:256]
            sel3 = dhdt[:, 256:260]
            WpW = dhdt[:, 260:388]  # (proj1_W @ pw_W[2]) block-diag

            # 4x-replicated moment basis via stride-0 broadcast copies; the
            # Vm halves land with the state, so DVE builds these while the
            # PE transposes run
            Vm4 = cp.tile([128, 2048], BF16, tag="Vm4")
            for half in range(2):
                nc.vector.tensor_copy(
                    Vm4[:, 1024 * half:1024 * (half + 1)]
                    .rearrange("p (t r k) -> p t r k", t=8, r=8),
                    Vmh[half].rearrange("p (t k) -> p t k", t=8)
                    .unsqueeze(2).broadcast_to((128, 8, 8, K)),
                )

            fq = None  # layer-0 state is read from fqh0

            def fq_block(m):
                if fq is None:
                    return fqh0[m // 2][:, 128 * (m % 2):128 * (m % 2 + 1)]
                return fq[:, 128 * m:128 * (m + 1)]

            def fq_half(h):
                if fq is None:
                    return fqh0[h]
                return fq[:, 256 * h:256 * (h + 1)]

            # ---------------- KNO layers ----------------
            g1 = None
            for i in range(DEPTH):
                last = i == DEPTH - 1
                # natural layout: trp[p, 128m + 32j + c] = fq[512j+128m+p, c]
                # -- one tile per half so the U-multiplies wait only on their
                # own half's transposes (dependency tracking is per-tile)
                trph = [
                    ptr.tile([128, 256], BF16, tag="trpA", name=f"trpA_{i}"),
                    ptr.tile([128, 256], BF16, tag="trpB", name=f"trpB_{i}"),
                ]
                # one PSUM bank per half: the accumulation group (skip /
                # folded-proj1 matmul, later joined by the integral via a PE
                # identity accumulate) stays open over the whole layer
                skph = [
                    pmix.tile([128, 512], F32, tag="mix", name=f"skpA_{i}"),
                    pmix.tile([128, 512], F32, tag="mix", name=f"skpB_{i}"),
                ]
                U = wp.tile([128, 512], BF16, tag="U")
                for m in range(4):
                    nc.tensor.transpose(
                        trph[m // 2][:, 128 * (m % 2):128 * (m % 2 + 1)],
                        fq_block(m),
                        ident[:],
                    )
                # U = fq_nat * (w e^{-a x^2}), halves so moments start early
                for h, (lo, hi) in enumerate(LSPLIT):
                    nc.vector.tensor_mul(
                        U[:, lo:hi], trph[h][:, 0:256], tmpN[i][:, lo:hi]
                    )
                # replicated basis -> M lands on all 4 partition groups
                Mp4 = pmom.tile([128, C], F32, tag="Mp4", name=f"Mp4_{i}")
                for t in range(16):
                    m, j = divmod(t, 4)
                    nc.tensor.matmul(
                        Mp4[:],
                        Vm4[:, 128 * t:128 * (t + 1)],
                        U[:, 128 * m + 32 * j:128 * m + 32 * j + 32],
                        start=(t == 0),
                        stop=(t == 15),
                    )
                # skip^T via block-diagonal weights in the PE idle slot here;
                # layer 2 folds proj1 in: pp1 = (proj1 pw2)^T fq1 + proj1^T z2
                for h, (lo, hi) in enumerate(LSPLIT):
                    nc.tensor.matmul(
                        skph[h][:, lo:hi], WpW if last else pwW[i],
                        fq_half(h), start=True, stop=False,
                    )
                nc.vector.tensor_mul(
                    Mt4[:].rearrange("p (r c) -> p r c", r=4),
                    Mp4[:].unsqueeze(1).broadcast_to((128, 4, C)),
                    B4M[i][:].rearrange("p (r c) -> p r c", r=4),
                )
                # eval + combine, pipelined in column halves:
                # fq_next = gelu(skip + pw_b + Es * (Mt4^T @ VTs))
                PT = pev.tile([128, 512], F32, tag="PT")
                z = wp.tile([128, 512], BF16, tag="z")
                fq2 = wp.tile([128, 512], BF16, tag="fq", name=f"fq2_{i}")
                for lo, hi in LSPLIT:
                    nc.tensor.matmul(
                        PT[:, lo:hi], Mt4[:], VTs[:, lo:hi], start=True, stop=True
                    )
                for h, (lo, hi) in enumerate(LSPLIT):
                    sl = slice(lo, hi)
                    nc.vector.tensor_mul(z[:, sl], PT[:, sl], EsT[i][:, sl])
                    # PE accumulate of the integral onto the open skip bank
                    # (identity for layers 0/1; proj1 mixing for layer 2),
                    # then gelu straight from PSUM
                    nc.tensor.matmul(
                        skph[h][:, sl], p1W if last else ident[:], z[:, sl],
                        start=False, stop=True,
                    )
                    nc.scalar.activation(
                        fq2[:, sl], skph[h][:, sl], AF.Gelu_apprx_tanh,
                        bias=p1b if last else pwb[i],
                    )
                fq = fq2
            g1 = fq  # = gelu(proj1 @ (skip2 + integ2) + p1b)

            # ------------- projection head tail (stage-major) ---------------
            # stage-major emission so the PE never runs a later stage of
            # chunk 0 ahead of an earlier stage of chunk 1
            pp2h = [
                pmix.tile([128, 512], F32, tag="mix", name="pp2A"),
                pmix.tile([128, 512], F32, tag="mix", name="pp2B"),
            ]
            g2 = wp.tile([128, 512], BF16, tag="fq")
            pouth = [
                pop.tile([4, 512], F32, tag="pout", name="poutA"),
                pop.tile([4, 512], F32, tag="pout", name="poutB"),
            ]
            outsb = wp.tile([4, 512], F32, tag="outsb")
            for h, (lo, hi) in enumerate(LSPLIT):
                sl = slice(lo, hi)
                nc.tensor.matmul(pp2h[h][:, sl], p2W, g1[:, sl], start=True, stop=True)
                nc.scalar.activation(
                    g2[:, sl], pp2h[h][:, sl], AF.Gelu_apprx_tanh, bias=p2b
                )
            for h, (lo, hi) in enumerate(LSPLIT):
                sl = slice(lo, hi)
                # final dot: proj3_W folded into a selection stationary; the
                # output bias rides the SBUF-staging Identity's bias port on
                # the now-idle ACT engine (Identity shares the gelu table set)
                nc.tensor.matmul(pouth[h][:, sl], sel3, g2[:, sl], start=True, stop=True)
                nc.scalar.activation(
                    outsb[:, sl], pouth[h][:, sl], AF.Identity, bias=b3c
                )
                eng = nc.gpsimd if lo == 0 else nc.sync
                eng.dma_start(out_dram[:, sl], outsb[:, sl])

    return nc


def get_nc():
    if "nc" not in _CACHE:
        nc = bacc.Bacc("TRN2", target_bir_lowering=False, debug=False, num_devices=NCORES)
        build_program(nc)
        nc.compile()
        _CACHE["nc"] = nc
    return _CACHE["nc"]


def make_in_map(
    f_x, x_grid, q_weights, lift_W, lift_b, pw_W, pw_b, ker_log_ell, ker_log_sigma,
    proj1_W, proj1_b, proj2_W, proj2_b, proj3_W, proj3_b,
):
    f8 = lambda a: np.asarray(a, dtype=np.float64)
    x = f8(x_grid).reshape(N)
    w = f8(q_weights).reshape(N)
    f = f8(f_x).reshape(N)
    a = 0.5 * np.exp(-2.0 * f8(ker_log_ell))          # [DEPTH, C]
    sig2 = np.exp(2.0 * f8(ker_log_sigma))            # [DEPTH, C]
    ks = np.arange(K, dtype=np.float64)
    lnfact = np.concatenate([[0.0], np.cumsum(np.log(np.arange(1, K)))])

    p = np.arange(128)
    npr = np.arange(512)

    # host lift: fqT[32j+c, n'] = gelu([f,x] @ lift_W.T + lift_b)[512j+n', c]
    pre = np.stack([f, x], axis=1) @ f8(lift_W).T + f8(lift_b)  # [N, C]
    fql = 0.5 * pre * (1.0 + np.tanh(
        0.7978845608028654 * (pre + 0.044715 * pre ** 3)))
    fqT = np.zeros((128, 512), np.float64)
    for j in range(4):
        fqT[32 * j:32 * (j + 1), :] = fql[512 * j:512 * (j + 1), :].T

    def bd(W):  # block-diag lhsT: [32j+c', 32j+c] = W[c, c']
        M = np.zeros((128, 128), np.float64)
        for j in range(4):
            M[32 * j:32 * (j + 1), 32 * j:32 * (j + 1)] = f8(W).T
        return M

    def tmpN_of(i):
        t = np.zeros((128, 512), np.float64)
        for m in range(4):
            for j in range(4):
                q = 512 * j + 128 * m + p
                t[:, 128 * m + 32 * j:128 * m + 32 * j + 32] = (
                    w[q, None] * np.exp(-a[i][None, :] * (x[q, None] ** 2)))
        return t

    def EsT_of(i):
        e = np.zeros((128, 512), np.float64)
        for j in range(4):
            nn = 512 * j + npr
            e[32 * j:32 * (j + 1), :] = (
                sig2[i][:, None] * np.exp(-a[i][:, None] * (x[None, nn] ** 2)))
        return e

    def B4M_of(i):
        # eval stationary mask: rows 16g+k (8 replication groups); only
        # group g==j contributes to output node-chunk j, rest stay zero
        M = np.zeros((128, 128), np.float64)
        B = np.exp(ks[:, None] * np.log(2.0 * a[i][None, :]) - lnfact[:, None])
        for j in range(4):
            M[K * j:K * (j + 1), 32 * j:32 * (j + 1)] = B
        return M

    # moment basis Vm[p, K(4m+j)+k] = x_{512j+128m+p}^k
    dvm = np.zeros((128, 16 * K), np.float64)
    for m in range(4):
        for j in range(4):
            q = 512 * j + 128 * m + p
            dvm[:, K * (4 * m + j):K * (4 * m + j + 1)] = x[q, None] ** ks[None, :]

    da = np.concatenate([fqT[:, 0:256], dvm[:, 0:8 * K]], axis=1)
    db = np.concatenate(
        [fqT[:, 256:512], dvm[:, 8 * K:16 * K], B4M_of(0)], axis=1)

    cfa = np.zeros((128, CF_W), np.float64)
    cfa[:, 0] = np.tile(f8(pw_b)[0], 4)
    cfa[:, 1] = np.tile(f8(pw_b)[1], 4)
    cfa[:, 2] = np.tile(f8(proj1_b) + f8(proj1_W) @ f8(pw_b)[2], 4)
    cfa[:, 3] = np.tile(f8(proj2_b), 4)
    cfa[0:4, 4] = f8(proj3_b)[0]
    cf16 = cfa.astype(np.float32).view(NPBF16)  # raw f32 halves as bf16 cols

    dgp = np.concatenate(
        [tmpN_of(0).astype(NPBF16), bd(pw_W[0]).astype(NPBF16), cf16], axis=1)

    dy = np.zeros((128, 512), np.float64)
    for j in range(4):
        dy[K * j:K * (j + 1), 0:512] = (
            x[None, 512 * j:512 * (j + 1)] ** ks[:, None])

    dhd = np.zeros((128, DHD_W), np.float64)
    dhd[:, 0:128] = bd(proj1_W)
    dhd[:, 128:256] = bd(proj2_W)
    for j in range(4):
        dhd[32 * j:32 * (j + 1), 256 + j] = f8(proj3_W)[0]
    dhd[:, 260:388] = bd(f8(proj1_W) @ f8(pw_W[2]))

    def dl_of(i):
        dl = np.zeros((128, DL_W), np.float64)
        dl[:, 0:512] = tmpN_of(i)
        dl[:, 512:1024] = EsT_of(i)
        dl[:, 1024:1152] = B4M_of(i)
        dl[:, 1152:1280] = bd(pw_W[i])
        return dl

    return {
        "da": da.astype(NPBF16),
        "db": db.astype(NPBF16),
        "dg": dgp,
        "dy": dy.astype(NPBF16),
        "de0": EsT_of(0).astype(NPBF16),
        "dhd": dhd.astype(NPBF16),
        "dl1": dl_of(1).astype(NPBF16),
        "dl2": dl_of(2).astype(NPBF16),
    }


def kernel(**inputs) -> np.ndarray:
    nc = get_nc()
    in_map = make_in_map(**inputs)
    res = run_bass_kernel_spmd(nc, [in_map] * NCORES, list(range(NCORES)))
    return np.asarray(res.results[0]["out"], dtype=np.float32).reshape(N)


# revision 104
# speedup vs baseline: 1.0646x; 1.0203x over previous
"""Trainium2 Bass kernel for a 1D Kernel Neural Operator (KNO) on a regular grid.

Reference computation (N=2048 nodes, C=32 channels, DEPTH=3):
    fq = gelu([f_x, x] @ lift_W.T + lift_b)
    for i in 0..2:
        skip  = fq @ pw_W[i].T + pw_b[i]
        K_c   = sig2_c * exp(-(x_n - x_q)^2 * a_c),  a_c = 1/(2*ell2_c)
        integ = einsum('cnq,qc->nc', K, fq * w)
        fq    = skip + integ; gelu if i < 2
    out = (gelu(gelu(fq@W1.T+b1)@W2.T+b2)) @ W3.T + b3

Instead of materializing the C x N x N kernels, we use the factorization
exp(-a(x_n-x_q)^2) = e^{-a x_n^2} e^{2 a x_n x_q} e^{-a x_q^2} with the Taylor
expansion e^{2a x_n x_q} = sum_k (2a)^k/k! x_n^k x_q^k (K=16 terms;
truncation err 6e-5, far below the bf16 quantization floor). Each
layer's integral is two matmuls through the moment basis V[n,k] = x_n^k:
    U       = fq ⊙ (w_q e^{-a_c x_q^2})             [N,C]
    M[k,c]  = sum_q V[q,k] U[q,c]                   [K,C]   (PE, 16 psum-accum)
    Mt      = M ⊙ B,  B[k,c] = (2a_c)^k/k!
    integ   = (s2_c e^{-a_c x_n^2}) ⊙ (VT.T @ Mt)   [N,C]   (PE)

All constant tensors (the gelu'd lift of the inputs, V, VT, the Gaussian
envelopes, B, block-diag mixing weights) are pure functions of the inputs and
are precomputed on the host; the device program is the KNO operator layers +
projection head. All matmul operands are bf16; PSUM accumulation stays fp32.

Data layout: the [N,C] state lives channel-transposed in a 4-chunk stack
fqT[32j + c, n'] = fq[512j + n', c], a single [128, 512] SBUF tile. Channel
mixing (pw/proj) is ONE matmul per layer with host-built block-diagonal
[128,128] weights; biases ride the gelu activation's per-partition bias port.
The moment contraction needs q on partitions, so each layer does 4 PE
transposes of fqT back to natural layout (identity built on-device with one
affine_select); per-half transpose tiles keep the dependency tracking exact.
Moment matmuls use an 8x-replicated basis (stride-0 broadcast DVE copies;
8 groups x K=16 keeps the stationaries 128 columns wide -- 64-wide
stationaries drop the PE into a slower half-column-group mode) so M lands
replicated across partition groups and one broadcast multiply against a
group-masked B4 builds the [128,128] eval stationary. The skip PSUM
banks stay open through layers 0/1: the integral joins them via a PE
identity-matmul accumulate and the gelu reads the sum straight from PSUM.
Layer 2 has no gelu, so proj1 distributes over it: pp1 = (proj1 pw2)^T fq1
(PE, early, accumulation group left open) + proj1^T z2 (PE accumulate) --
layer 2's skip matmul, combine-add and the standalone proj1 matmul all
disappear from the chain.

DMA scheduling: the kernel is LATENCY-bound -- each DMA ring slot lands
~2us after the previous one almost regardless of size, so the layer-0
critical tensors are MERGED into the three slot-1 packs:
    sync  slot1: fqT | Vm (half 0)          slot2: VTs    slot3: dl2
    act   slot1: fqT | Vm (half 1) | B4M0   slot2: dl1    slot3: dhd
    gpsimd slot1: tmpN0 | pwW0 | cf (f32 bitcast)  slot2: EsT0
eval/combine/gelu are split into 256-column halves so PE/DVE/ACT pipeline
across halves; the skip matmuls fill PE idle slots during the transposes.

Sharding: the whole problem is a dependent chain of small ops, so all 8 cores
run identical replicas (collectives cost more than they save); the output is
taken from core 0.
"""

import numpy as np
import ml_dtypes

import concourse.bass as bass
import concourse.tile as tile
from concourse import bacc, mybir
from concourse.bass_utils import run_bass_kernel_spmd

N = 2048
C = 32
K = 16   # Taylor terms: fp64 truncation err 6e-5, far below the bf16 floor
DEPTH = 3
NCORES = 8
F32 = mybir.dt.float32
BF16 = mybir.dt.bfloat16
AF = mybir.ActivationFunctionType
ALU = mybir.AluOpType
NPBF16 = ml_dtypes.bfloat16

DA_W = 640     # fqT half | Vm half | tmpN0 half
DB_W = 640     # fqT half | Vm half | tmpN0 half
DG_W = 266     # B4M0 + pwW0 + cf (5 x f32 as 10 bf16 cols)
DE0_W = 512    # EsT0
DHD_W = 388    # p1W + p2W + sel3 + (proj1 pw2) block-diag
DL_W = 1280    # per-layer: tmpN + EsT + B4M + pwW  (layers 1, 2)
CF_W = 5
LSPLIT = [(0, 256), (256, 512)]  # layer halves

_CACHE = {}


def build_program(nc):
    da = nc.dram_tensor("da", [128, DA_W], BF16, kind="ExternalInput")
    db = nc.dram_tensor("db", [128, DB_W], BF16, kind="ExternalInput")
    dg = nc.dram_tensor("dg", [128, DG_W], BF16, kind="ExternalInput")
    dy = nc.dram_tensor("dy", [128, 512], BF16, kind="ExternalInput")
    de0 = nc.dram_tensor("de0", [128, DE0_W], BF16, kind="ExternalInput")
    dhd = nc.dram_tensor("dhd", [128, DHD_W], BF16, kind="ExternalInput")
    dl1 = nc.dram_tensor("dl1", [128, DL_W], BF16, kind="ExternalInput")
    dl2 = nc.dram_tensor("dl2", [128, DL_W], BF16, kind="ExternalInput")
    out_dram = nc.dram_tensor("out", [4, 512], F32, kind="ExternalOutput")

    with tile.TileContext(nc) as tc:
        with (
            tc.tile_pool(name="const", bufs=1) as cp,
            tc.tile_pool(name="work", bufs=3) as wp,
            tc.tile_pool(name="pmix", bufs=2, space="PSUM") as pmix,
            tc.tile_pool(name="ptr", bufs=1, space="PSUM") as ptr,
            tc.tile_pool(name="pmom", bufs=1, space="PSUM") as pmom,
            tc.tile_pool(name="pev", bufs=1, space="PSUM") as pev,
            tc.tile_pool(name="pout", bufs=2, space="PSUM") as pop,
        ):
            # warm-act source and home of the eval stationary
            Mt4 = cp.tile([128, 128], BF16, tag="Mt4")
            nc.vector.memset(Mt4[:], 0.0)

            # slot-1 packs: everything layer 0 touches early
            dat = cp.tile([128, DA_W], BF16, tag="dat")
            nc.sync.dma_start(dat[:], da[:])
            dyt = cp.tile([128, 512], BF16, tag="dyt")
            nc.sync.dma_start(dyt[:], dy[:])
            dl2t = cp.tile([128, DL_W], BF16, tag="dl2t")
            nc.sync.dma_start(dl2t[:], dl2[:])

            dbt = cp.tile([128, DB_W], BF16, tag="dbt")
            nc.scalar.dma_start(dbt[:], db[:])

            dgt = cp.tile([128, DG_W], BF16, tag="dgt")
            nc.gpsimd.dma_start(dgt[:], dg[:])
            de0t = cp.tile([128, DE0_W], BF16, tag="de0t")
            nc.gpsimd.dma_start(de0t[:], de0[:])
            # on-device transpose identity: ones tile + diagonal select
            # (issued after both gpsimd DMAs; still ready before the first
            # transpose, which waits on the slot-1 state pack)
            ident = cp.tile([128, 128], BF16, tag="ident")
            nc.gpsimd.memset(ident[:], 1.0)
            nc.gpsimd.affine_select(
                ident[:], ident[:], [[-1, 128]],
                ALU.is_equal, 0.0, base=0, channel_multiplier=1,
            )

            # prefetch the gelu activation table during the DMA wait
            warm = wp.tile([1, 8], BF16, tag="warm")
            nc.scalar.activation(warm[:], Mt4[0:1, 0:8], AF.Gelu_apprx_tanh)

            dl1t = cp.tile([128, DL_W], BF16, tag="dl1t")
            nc.scalar.dma_start(dl1t[:], dl1[:])
            dhdt = cp.tile([128, DHD_W], BF16, tag="dhdt")
            nc.scalar.dma_start(dhdt[:], dhd[:])

            # accessor lists; layer-0 sources live split across the two
            # slot-1 packs (half h in dat/dbt)
            fqh0 = [dat[:, 0:256], dbt[:, 0:256]]
            Vmh = [dat[:, 256:256 + 8 * K], dbt[:, 256:256 + 8 * K]]
            tmpN0h = [dat[:, 384:640], dbt[:, 384:640]]
            tmpN = [None, dl1t[:, 0:512], dl2t[:, 0:512]]
            EsT = [de0t[:, 0:512], dl1t[:, 512:1024], dl2t[:, 512:1024]]
            VTs = dyt[:, 0:512]
            pwW = [dgt[:, 128:256], dl1t[:, 1152:1280], None]
            B4M = [dgt[:, 0:128], dl1t[:, 1024:1152], dl2t[:, 1024:1152]]
            cfv = dgt[:, 256:266].bitcast(F32)
            pwb = [cfv[:, 0:1], cfv[:, 1:2]]
            p1b = cfv[:, 2:3]
            p2b = cfv[:, 3:4]
            b3c = cfv[0:4, 4:5]
            p1W = dhdt[:, 0:128]
            p2W = dhdt[:, 128:256]
            sel3 = dhdt[:, 256:260]
            WpW = dhdt[:, 260:388]  # (proj1_W @ pw_W[2]) block-diag

            # 4x-replicated moment basis via stride-0 broadcast copies; the
            # Vm halves land with the state, so DVE builds these while the
            # PE transposes run
            Vm4 = cp.tile([128, 2048], BF16, tag="Vm4")
            for half in range(2):
                nc.vector.tensor_copy(
                    Vm4[:, 1024 * half:1024 * (half + 1)]
                    .rearrange("p (t r k) -> p t r k", t=8, r=8),
                    Vmh[half].rearrange("p (t k) -> p t k", t=8)
                    .unsqueeze(2).broadcast_to((128, 8, 8, K)),
                )

            fq = None  # layer-0 state is read from fqh0

            def fq_block(m):
                if fq is None:
                    return fqh0[m // 2][:, 128 * (m % 2):128 * (m % 2 + 1)]
                return fq[:, 128 * m:128 * (m + 1)]

            def fq_half(h):
                if fq is None:
                    return fqh0[h]
                return fq[:, 256 * h:256 * (h + 1)]

            # ---------------- KNO layers ----------------
            g1 = None
            for i in range(DEPTH):
                last = i == DEPTH - 1
                # natural layout: trp[p, 128m + 32j + c] = fq[512j+128m+p, c]
                # -- one tile per half so the U-multiplies wait only on their
                # own half's transposes (dependency tracking is per-tile)
                trph = [
                    ptr.tile([128, 256], BF16, tag="trpA", name=f"trpA_{i}"),
                    ptr.tile([128, 256], BF16, tag="trpB", name=f"trpB_{i}"),
                ]
                # one PSUM bank per half: the accumulation group (skip /
                # folded-proj1 matmul, later joined by the integral via a PE
                # identity accumulate) stays open over the whole layer
                skph = [
                    pmix.tile([128, 512], F32, tag="mix", name=f"skpA_{i}"),
                    pmix.tile([128, 512], F32, tag="mix", name=f"skpB_{i}"),
                ]
                U = wp.tile([128, 512], BF16, tag="U")
                for m in range(4):
                    nc.tensor.transpose(
                        trph[m // 2][:, 128 * (m % 2):128 * (m % 2 + 1)],
                        fq_block(m),
                        ident[:],
                    )
                # U = fq_nat * (w e^{-a x^2}), halves so moments start early
                for h, (lo, hi) in enumerate(LSPLIT):
                    tn = tmpN0h[h] if i == 0 else tmpN[i][:, lo:hi]
                    nc.vector.tensor_mul(U[:, lo:hi], trph[h][:, 0:256], tn)
                # replicated basis -> M lands on all 4 partition groups
                Mp4 = pmom.tile([128, C], F32, tag="Mp4", name=f"Mp4_{i}")
                for t in range(16):
                    m, j = divmod(t, 4)
                    nc.tensor.matmul(
                        Mp4[:],
                        Vm4[:, 128 * t:128 * (t + 1)],
                        U[:, 128 * m + 32 * j:128 * m + 32 * j + 32],
                        start=(t == 0),
                        stop=(t == 15),
                    )
                # skip^T via block-diagonal weights in the PE idle slot here;
                # layer 2 folds proj1 in: pp1 = (proj1 pw2)^T fq1 + proj1^T z2
                for h, (lo, hi) in enumerate(LSPLIT):
                    nc.tensor.matmul(
                        skph[h][:, lo:hi], WpW if last else pwW[i],
                        fq_half(h), start=True, stop=False,
                    )
                nc.vector.tensor_mul(
                    Mt4[:].rearrange("p (r c) -> p r c", r=4),
                    Mp4[:].unsqueeze(1).broadcast_to((128, 4, C)),
                    B4M[i][:].rearrange("p (r c) -> p r c", r=4),
                )
                # eval + combine, pipelined in column halves:
                # fq_next = gelu(skip + pw_b + Es * (Mt4^T @ VTs))
                PT = pev.tile([128, 512], F32, tag="PT")
                z = wp.tile([128, 512], BF16, tag="z")
                fq2 = wp.tile([128, 512], BF16, tag="fq", name=f"fq2_{i}")
                for lo, hi in LSPLIT:
                    nc.tensor.matmul(
                        PT[:, lo:hi], Mt4[:], VTs[:, lo:hi], start=True, stop=True
                    )
                for h, (lo, hi) in enumerate(LSPLIT):
                    sl = slice(lo, hi)
                    nc.vector.tensor_mul(z[:, sl], PT[:, sl], EsT[i][:, sl])
                    # PE accumulate of the integral onto the open skip bank
                    # (identity for layers 0/1; proj1 mixing for layer 2),
                    # then gelu straight from PSUM
                    nc.tensor.matmul(
                        skph[h][:, sl], p1W if last else ident[:], z[:, sl],
                        start=False, stop=True,
                    )
                    nc.scalar.activation(
                        fq2[:, sl], skph[h][:, sl], AF.Gelu_apprx_tanh,
                        bias=p1b if last else pwb[i],
                    )
                fq = fq2
            g1 = fq  # = gelu(proj1 @ (skip2 + integ2) + p1b)

            # ------------- projection head tail (stage-major) ---------------
            # stage-major emission so the PE never runs a later stage of
            # chunk 0 ahead of an earlier stage of chunk 1
            pp2h = [
                pmix.tile([128, 512], F32, tag="mix", name="pp2A"),
                pmix.tile([128, 512], F32, tag="mix", name="pp2B"),
            ]
            g2 = wp.tile([128, 512], BF16, tag="fq")
            pouth = [
                pop.tile([4, 512], F32, tag="pout", name="poutA"),
                pop.tile([4, 512], F32, tag="pout", name="poutB"),
            ]
            outsb = wp.tile([4, 512], F32, tag="outsb")
            for h, (lo, hi) in enumerate(LSPLIT):
                sl = slice(lo, hi)
                nc.tensor.matmul(pp2h[h][:, sl], p2W, g1[:, sl], start=True, stop=True)
                nc.scalar.activation(
                    g2[:, sl], pp2h[h][:, sl], AF.Gelu_apprx_tanh, bias=p2b
                )
            for h, (lo, hi) in enumerate(LSPLIT):
                sl = slice(lo, hi)
                # final dot: proj3_W folded into a selection stationary; the
                # output bias rides the SBUF-staging Identity's bias port on
                # the now-idle ACT engine (Identity shares the gelu table set)
                nc.tensor.matmul(pouth[h][:, sl], sel3, g2[:, sl], start=True, stop=True)
                nc.scalar.activation(
                    outsb[:, sl], pouth[h][:, sl], AF.Identity, bias=b3c
                )
                eng = nc.gpsimd if lo == 0 else nc.sync
                eng.dma_start(out_dram[:, sl], outsb[:, sl])

    return nc


def get_nc():
    if "nc" not in _CACHE:
        nc = bacc.Bacc("TRN2", target_bir_lowering=False, debug=False, num_devices=NCORES)
        build_program(nc)
        nc.compile()
        _CACHE["nc"] = nc
    return _CACHE["nc"]


def make_in_map(
    f_x, x_grid, q_weights, lift_W, lift_b, pw_W, pw_b, ker_log_ell, ker_log_sigma,
    proj1_W, proj1_b, proj2_W, proj2_b, proj3_W, proj3_b,
):
    f8 = lambda a: np.asarray(a, dtype=np.float64)
    x = f8(x_grid).reshape(N)
    w = f8(q_weights).reshape(N)
    f = f8(f_x).reshape(N)
    a = 0.5 * np.exp(-2.0 * f8(ker_log_ell))          # [DEPTH, C]
    sig2 = np.exp(2.0 * f8(ker_log_sigma))            # [DEPTH, C]
    ks = np.arange(K, dtype=np.float64)
    lnfact = np.concatenate([[0.0], np.cumsum(np.log(np.arange(1, K)))])

    p = np.arange(128)
    npr = np.arange(512)

    # host lift: fqT[32j+c, n'] = gelu([f,x] @ lift_W.T + lift_b)[512j+n', c]
    pre = np.stack([f, x], axis=1) @ f8(lift_W).T + f8(lift_b)  # [N, C]
    fql = 0.5 * pre * (1.0 + np.tanh(
        0.7978845608028654 * (pre + 0.044715 * pre ** 3)))
    fqT = np.zeros((128, 512), np.float64)
    for j in range(4):
        fqT[32 * j:32 * (j + 1), :] = fql[512 * j:512 * (j + 1), :].T

    def bd(W):  # block-diag lhsT: [32j+c', 32j+c] = W[c, c']
        M = np.zeros((128, 128), np.float64)
        for j in range(4):
            M[32 * j:32 * (j + 1), 32 * j:32 * (j + 1)] = f8(W).T
        return M

    def tmpN_of(i):
        t = np.zeros((128, 512), np.float64)
        for m in range(4):
            for j in range(4):
                q = 512 * j + 128 * m + p
                t[:, 128 * m + 32 * j:128 * m + 32 * j + 32] = (
                    w[q, None] * np.exp(-a[i][None, :] * (x[q, None] ** 2)))
        return t

    def EsT_of(i):
        e = np.zeros((128, 512), np.float64)
        for j in range(4):
            nn = 512 * j + npr
            e[32 * j:32 * (j + 1), :] = (
                sig2[i][:, None] * np.exp(-a[i][:, None] * (x[None, nn] ** 2)))
        return e

    def B4M_of(i):
        # eval stationary mask: rows 16g+k (8 replication groups); only
        # group g==j contributes to output node-chunk j, rest stay zero
        M = np.zeros((128, 128), np.float64)
        B = np.exp(ks[:, None] * np.log(2.0 * a[i][None, :]) - lnfact[:, None])
        for j in range(4):
            M[K * j:K * (j + 1), 32 * j:32 * (j + 1)] = B
        return M

    # moment basis Vm[p, K(4m+j)+k] = x_{512j+128m+p}^k
    dvm = np.zeros((128, 16 * K), np.float64)
    for m in range(4):
        for j in range(4):
            q = 512 * j + 128 * m + p
            dvm[:, K * (4 * m + j):K * (4 * m + j + 1)] = x[q, None] ** ks[None, :]

    tmpN0 = tmpN_of(0)
    da = np.concatenate(
        [fqT[:, 0:256], dvm[:, 0:8 * K], tmpN0[:, 0:256]], axis=1)
    db = np.concatenate(
        [fqT[:, 256:512], dvm[:, 8 * K:16 * K], tmpN0[:, 256:512]], axis=1)

    cfa = np.zeros((128, CF_W), np.float64)
    cfa[:, 0] = np.tile(f8(pw_b)[0], 4)
    cfa[:, 1] = np.tile(f8(pw_b)[1], 4)
    cfa[:, 2] = np.tile(f8(proj1_b) + f8(proj1_W) @ f8(pw_b)[2], 4)
    cfa[:, 3] = np.tile(f8(proj2_b), 4)
    cfa[0:4, 4] = f8(proj3_b)[0]
    cf16 = cfa.astype(np.float32).view(NPBF16)  # raw f32 halves as bf16 cols

    dgp = np.concatenate(
        [B4M_of(0).astype(NPBF16), bd(pw_W[0]).astype(NPBF16), cf16], axis=1)

    dy = np.zeros((128, 512), np.float64)
    for j in range(4):
        dy[K * j:K * (j + 1), 0:512] = (
            x[None, 512 * j:512 * (j + 1)] ** ks[:, None])

    dhd = np.zeros((128, DHD_W), np.float64)
    dhd[:, 0:128] = bd(proj1_W)
    dhd[:, 128:256] = bd(proj2_W)
    for j in range(4):
        dhd[32 * j:32 * (j + 1), 256 + j] = f8(proj3_W)[0]
    dhd[:, 260:388] = bd(f8(proj1_W) @ f8(pw_W[2]))

    def dl_of(i):
        dl = np.zeros((128, DL_W), np.float64)
        dl[:, 0:512] = tmpN_of(i)
        dl[:, 512:1024] = EsT_of(i)
        dl[:, 1024:1152] = B4M_of(i)
        dl[:, 1152:1280] = bd(pw_W[i])
        return dl

    return {
        "da": da.astype(NPBF16),
        "db": db.astype(NPBF16),
        "dg": dgp,
        "dy": dy.astype(NPBF16),
        "de0": EsT_of(0).astype(NPBF16),
        "dhd": dhd.astype(NPBF16),
        "dl1": dl_of(1).astype(NPBF16),
        "dl2": dl_of(2).astype(NPBF16),
    }


def kernel(**inputs) -> np.ndarray:
    nc = get_nc()
    in_map = make_in_map(**inputs)
    res = run_bass_kernel_spmd(nc, [in_map] * NCORES, list(range(NCORES)))
    return np.asarray(res.results[0]["out"], dtype=np.float32).reshape(N)


# revision 107
# speedup vs baseline: 1.0756x; 1.0104x over previous
"""Trainium2 Bass kernel for a 1D Kernel Neural Operator (KNO) on a regular grid.

Reference computation (N=2048 nodes, C=32 channels, DEPTH=3):
    fq = gelu([f_x, x] @ lift_W.T + lift_b)
    for i in 0..2:
        skip  = fq @ pw_W[i].T + pw_b[i]
        K_c   = sig2_c * exp(-(x_n - x_q)^2 * a_c),  a_c = 1/(2*ell2_c)
        integ = einsum('cnq,qc->nc', K, fq * w)
        fq    = skip + integ; gelu if i < 2
    out = (gelu(gelu(fq@W1.T+b1)@W2.T+b2)) @ W3.T + b3

Instead of materializing the C x N x N kernels, we use the factorization
exp(-a(x_n-x_q)^2) = e^{-a x_n^2} e^{2 a x_n x_q} e^{-a x_q^2} with the Taylor
expansion e^{2a x_n x_q} = sum_k (2a)^k/k! x_n^k x_q^k (K=16 terms;
truncation err 6e-5, far below the bf16 quantization floor). Each
layer's integral is two matmuls through the moment basis V[n,k] = x_n^k:
    U       = fq ⊙ (w_q e^{-a_c x_q^2})             [N,C]
    M[k,c]  = sum_q V[q,k] U[q,c]                   [K,C]   (PE, 16 psum-accum)
    Mt      = M ⊙ B,  B[k,c] = (2a_c)^k/k!
    integ   = (s2_c e^{-a_c x_n^2}) ⊙ (VT.T @ Mt)   [N,C]   (PE)

All constant tensors (the gelu'd lift of the inputs, V, VT, the Gaussian
envelopes, B, block-diag mixing weights) are pure functions of the inputs and
are precomputed on the host; the device program is the KNO operator layers +
projection head. All matmul operands are bf16; PSUM accumulation stays fp32.

Data layout: the [N,C] state lives channel-transposed in a 4-chunk stack
fqT[32j + c, n'] = fq[512j + n', c], a single [128, 512] SBUF tile. Channel
mixing (pw/proj) is ONE matmul per layer with host-built block-diagonal
[128,128] weights; biases ride the gelu activation's per-partition bias port.
The moment contraction needs q on partitions, so each layer does 4 PE
transposes of fqT back to natural layout (identity built on-device with one
affine_select); per-half transpose tiles keep the dependency tracking exact.
Moment matmuls use an 8x-replicated basis (stride-0 broadcast DVE copies;
8 groups x K=16 keeps the stationaries 128 columns wide -- 64-wide
stationaries drop the PE into a slower half-column-group mode) so M lands
replicated across partition groups and one broadcast multiply against a
group-masked B4 builds the [128,128] eval stationary. The skip PSUM
banks stay open through layers 0/1: the integral joins them via a PE
identity-matmul accumulate and the gelu reads the sum straight from PSUM.
Layer 2 has no gelu, so proj1 distributes over it: pp1 = (proj1 pw2)^T fq1
(PE, early, accumulation group left open) + proj1^T z2 (PE accumulate) --
layer 2's skip matmul, combine-add and the standalone proj1 matmul all
disappear from the chain.

DMA scheduling: the kernel is LATENCY-bound -- each DMA ring slot lands
~2us after the previous one almost regardless of size, so the layer-0
critical tensors are MERGED into the three slot-1 packs:
    sync  slot1: fqT | Vm (half 0)          slot2: VTs    slot3: dl2
    act   slot1: fqT | Vm (half 1) | B4M0   slot2: dl1    slot3: dhd
    gpsimd slot1: tmpN0 | pwW0 | cf (f32 bitcast)  slot2: EsT0
eval/combine/gelu are split into 256-column halves so PE/DVE/ACT pipeline
across halves; the skip matmuls fill PE idle slots during the transposes.

Sharding: the whole problem is a dependent chain of small ops, so all 8 cores
run identical replicas (collectives cost more than they save); the output is
taken from core 0.
"""

import numpy as np
import ml_dtypes

import concourse.bass as bass
import concourse.tile as tile
from concourse import bacc, mybir
from concourse.bass_utils import run_bass_kernel_spmd

N = 2048
C = 32
K = 16   # Taylor terms: fp64 truncation err 6e-5, far below the bf16 floor
DEPTH = 3
NCORES = 8
F32 = mybir.dt.float32
BF16 = mybir.dt.bfloat16
AF = mybir.ActivationFunctionType
ALU = mybir.AluOpType
NPBF16 = ml_dtypes.bfloat16

DA_W = 640     # fqT half | Vm half | tmpN0 half
DB_W = 640     # fqT half | Vm half | tmpN0 half
DG_W = 266     # B4M0 + pwW0 + cf (5 x f32 as 10 bf16 cols)
DE0_W = 512    # EsT0
DHD_W = 388    # p1W + p2W + sel3 + (proj1 pw2) block-diag
DL_W = 1280    # per-layer: tmpN + EsT + B4M + pwW  (layers 1, 2)
CF_W = 5
LSPLIT = [(0, 256), (256, 512)]  # layer halves

_CACHE = {}


def build_program(nc):
    da = nc.dram_tensor("da", [128, DA_W], BF16, kind="ExternalInput")
    db = nc.dram_tensor("db", [128, DB_W], BF16, kind="ExternalInput")
    dg = nc.dram_tensor("dg", [128, DG_W], BF16, kind="ExternalInput")
    dy = nc.dram_tensor("dy", [128, 512], BF16, kind="ExternalInput")
    de0 = nc.dram_tensor("de0", [128, DE0_W], BF16, kind="ExternalInput")
    dhd = nc.dram_tensor("dhd", [128, DHD_W], BF16, kind="ExternalInput")
    dl1 = nc.dram_tensor("dl1", [128, DL_W], BF16, kind="ExternalInput")
    dl2 = nc.dram_tensor("dl2", [128, DL_W], BF16, kind="ExternalInput")
    out_dram = nc.dram_tensor("out", [4, 512], F32, kind="ExternalOutput")

    with tile.TileContext(nc) as tc:
        with (
            tc.tile_pool(name="const", bufs=1) as cp,
            tc.tile_pool(name="work", bufs=3) as wp,
            tc.tile_pool(name="pmix", bufs=2, space="PSUM") as pmix,
            tc.tile_pool(name="ptr", bufs=1, space="PSUM") as ptr,
            tc.tile_pool(name="pmom", bufs=1, space="PSUM") as pmom,
            tc.tile_pool(name="pev", bufs=1, space="PSUM") as pev,
            tc.tile_pool(name="pout", bufs=2, space="PSUM") as pop,
        ):
            # warm-act source and home of the eval stationary
            Mt4 = cp.tile([128, 128], BF16, tag="Mt4")
            nc.vector.memset(Mt4[:], 0.0)

            # slot-1 packs: everything layer 0 touches early
            dat = cp.tile([128, DA_W], BF16, tag="dat")
            nc.sync.dma_start(dat[:], da[:])
            dyt = cp.tile([128, 512], BF16, tag="dyt")
            nc.sync.dma_start(dyt[:], dy[:])
            dl2t = cp.tile([128, DL_W], BF16, tag="dl2t")
            nc.sync.dma_start(dl2t[:], dl2[:])

            dbt = cp.tile([128, DB_W], BF16, tag="dbt")
            nc.scalar.dma_start(dbt[:], db[:])

            dgt = cp.tile([128, DG_W], BF16, tag="dgt")
            nc.gpsimd.dma_start(dgt[:], dg[:])
            de0t = cp.tile([128, DE0_W], BF16, tag="de0t")
            nc.gpsimd.dma_start(de0t[:], de0[:])
            # on-device transpose identity: ones tile + diagonal select
            # (issued after both gpsimd DMAs; still ready before the first
            # transpose, which waits on the slot-1 state pack)
            ident = cp.tile([128, 128], BF16, tag="ident")
            nc.gpsimd.memset(ident[:], 1.0)
            nc.gpsimd.affine_select(
                ident[:], ident[:], [[-1, 128]],
                ALU.is_equal, 0.0, base=0, channel_multiplier=1,
            )

            # prefetch the gelu activation table during the DMA wait
            warm = wp.tile([1, 8], BF16, tag="warm")
            nc.scalar.activation(warm[:], Mt4[0:1, 0:8], AF.Gelu_apprx_tanh)

            dl1t = cp.tile([128, DL_W], BF16, tag="dl1t")
            nc.scalar.dma_start(dl1t[:], dl1[:])
            dhdt = cp.tile([128, DHD_W], BF16, tag="dhdt")
            nc.scalar.dma_start(dhdt[:], dhd[:])

            # accessor lists; layer-0 sources live split across the two
            # slot-1 packs (half h in dat/dbt)
            fqh0 = [dat[:, 0:256], dbt[:, 0:256]]
            Vmh = [dat[:, 256:256 + 8 * K], dbt[:, 256:256 + 8 * K]]
            tmpN0h = [dat[:, 384:640], dbt[:, 384:640]]
            tmpN = [None, dl1t[:, 0:512], dl2t[:, 0:512]]
            EsT = [de0t[:, 0:512], dl1t[:, 512:1024], dl2t[:, 512:1024]]
            VTs = dyt[:, 0:512]
            pwW = [dgt[:, 128:256], dl1t[:, 1152:1280], None]
            B4M = [dgt[:, 0:128], dl1t[:, 1024:1152], dl2t[:, 1024:1152]]
            cfv = dgt[:, 256:266].bitcast(F32)
            pwb = [cfv[:, 0:1], cfv[:, 1:2]]
            p1b = cfv[:, 2:3]
            p2b = cfv[:, 3:4]
            b3c = cfv[0:4, 4:5]
            p1W = dhdt[:, 0:128]
            p2W = dhdt[:, 128:256]
            sel3 = dhdt[:, 256:260]
            WpW = dhdt[:, 260:388]  # (proj1_W @ pw_W[2]) block-diag

            # 4x-replicated moment basis via stride-0 broadcast copies; the
            # Vm halves land with the state, so DVE builds these while the
            # PE transposes run
            Vm4 = cp.tile([128, 2048], BF16, tag="Vm4")
            for half in range(2):
                nc.vector.tensor_copy(
                    Vm4[:, 1024 * half:1024 * (half + 1)]
                    .rearrange("p (t r k) -> p t r k", t=8, r=8),
                    Vmh[half].rearrange("p (t k) -> p t k", t=8)
                    .unsqueeze(2).broadcast_to((128, 8, 8, K)),
                )

            fq = None  # layer-0 state is read from fqh0

            def fq_block(m):
                if fq is None:
                    return fqh0[m // 2][:, 128 * (m % 2):128 * (m % 2 + 1)]
                return fq[:, 128 * m:128 * (m + 1)]

            def fq_half(h):
                if fq is None:
                    return fqh0[h]
                return fq[:, 256 * h:256 * (h + 1)]

            # ---------------- KNO layers ----------------
            g1 = None
            for i in range(DEPTH):
                last = i == DEPTH - 1
                # natural layout: trp[p, 128m + 32j + c] = fq[512j+128m+p, c]
                # -- one tile per half so the U-multiplies wait only on their
                # own half's transposes (dependency tracking is per-tile)
                trph = [
                    ptr.tile([128, 256], BF16, tag="trpA", name=f"trpA_{i}"),
                    ptr.tile([128, 256], BF16, tag="trpB", name=f"trpB_{i}"),
                ]
                # one PSUM bank per half: the accumulation group (skip /
                # folded-proj1 matmul, later joined by the integral via a PE
                # identity accumulate) stays open over the whole layer
                skph = [
                    pmix.tile([128, 512], F32, tag="mix", name=f"skpA_{i}"),
                    pmix.tile([128, 512], F32, tag="mix", name=f"skpB_{i}"),
                ]
                U = wp.tile([128, 512], BF16, tag="U")
                for m in range(4):
                    nc.tensor.transpose(
                        trph[m // 2][:, 128 * (m % 2):128 * (m % 2 + 1)],
                        fq_block(m),
                        ident[:],
                    )
                # U = fq_nat * (w e^{-a x^2}), halves so moments start early
                for h, (lo, hi) in enumerate(LSPLIT):
                    tn = tmpN0h[h] if i == 0 else tmpN[i][:, lo:hi]
                    nc.vector.tensor_mul(U[:, lo:hi], trph[h][:, 0:256], tn)
                # replicated basis -> M lands on all 4 partition groups
                Mp4 = pmom.tile([128, C], F32, tag="Mp4", name=f"Mp4_{i}")
                for t in range(16):
                    m, j = divmod(t, 4)
                    nc.tensor.matmul(
                        Mp4[:],
                        Vm4[:, 128 * t:128 * (t + 1)],
                        U[:, 128 * m + 32 * j:128 * m + 32 * j + 32],
                        start=(t == 0),
                        stop=(t == 15),
                    )
                # skip^T via block-diagonal weights in the PE idle slot here;
                # layer 2 folds proj1 in: pp1 = (proj1 pw2)^T fq1 + proj1^T z2
                for h, (lo, hi) in enumerate(LSPLIT):
                    nc.tensor.matmul(
                        skph[h][:, lo:hi], WpW if last else pwW[i],
                        fq_half(h), start=True, stop=False,
                    )
                nc.vector.tensor_mul(
                    Mt4[:].rearrange("p (r c) -> p r c", r=4),
                    Mp4[:].unsqueeze(1).broadcast_to((128, 4, C)),
                    B4M[i][:].rearrange("p (r c) -> p r c", r=4),
                )
                # eval + combine, pipelined in column halves:
                # fq_next = gelu(skip + pw_b + Es * (Mt4^T @ VTs))
                PT = pev.tile([128, 512], F32, tag="PT")
                z = wp.tile([128, 512], BF16, tag="z")
                fq2 = wp.tile([128, 512], BF16, tag="fq", name=f"fq2_{i}")
                for lo, hi in LSPLIT:
                    nc.tensor.matmul(
                        PT[:, lo:hi], Mt4[:], VTs[:, lo:hi], start=True, stop=True
                    )
                for h, (lo, hi) in enumerate(LSPLIT):
                    sl = slice(lo, hi)
                    nc.vector.tensor_mul(z[:, sl], PT[:, sl], EsT[i][:, sl])
                    # PE accumulate of the integral onto the open skip bank
                    # (identity for layers 0/1; proj1 mixing for layer 2),
                    # then gelu straight from PSUM
                    nc.tensor.matmul(
                        skph[h][:, sl], p1W if last else ident[:], z[:, sl],
                        start=False, stop=True,
                    )
                    nc.scalar.activation(
                        fq2[:, sl], skph[h][:, sl], AF.Gelu_apprx_tanh,
                        bias=p1b if last else pwb[i],
                    )
                fq = fq2
            g1 = fq  # = gelu(proj1 @ (skip2 + integ2) + p1b)

            # ------------- projection head tail (stage-major) ---------------
            # stage-major emission so the PE never runs a later stage of
            # chunk 0 ahead of an earlier stage of chunk 1
            pp2h = [
                pmix.tile([128, 512], F32, tag="mix", name="pp2A"),
                pmix.tile([128, 512], F32, tag="mix", name="pp2B"),
            ]
            g2 = wp.tile([128, 512], BF16, tag="fq")
            pouth = [
                pop.tile([4, 512], F32, tag="pout", name="poutA"),
                pop.tile([4, 512], F32, tag="pout", name="poutB"),
            ]
            outsb = wp.tile([4, 512], F32, tag="outsb")
            for h, (lo, hi) in enumerate(LSPLIT):
                sl = slice(lo, hi)
                nc.tensor.matmul(pp2h[h][:, sl], p2W, g1[:, sl], start=True, stop=True)
                nc.scalar.activation(
                    g2[:, sl], pp2h[h][:, sl], AF.Gelu_apprx_tanh, bias=p2b
                )
            for h, (lo, hi) in enumerate(LSPLIT):
                sl = slice(lo, hi)
                # final dot: proj3_W folded into a selection stationary; the
                # output bias rides the SBUF-staging Identity's bias port on
                # the now-idle ACT engine (Identity shares the gelu table set)
                nc.tensor.matmul(pouth[h][:, sl], sel3, g2[:, sl], start=True, stop=True)
                nc.scalar.activation(
                    outsb[:, sl], pouth[h][:, sl], AF.Identity, bias=b3c
                )
                eng = nc.gpsimd if lo == 0 else nc.sync
                eng.dma_start(out_dram[:, sl], outsb[:, sl])

    return nc


def get_nc():
    if "nc" not in _CACHE:
        nc = bacc.Bacc("TRN2", target_bir_lowering=False, debug=False, num_devices=NCORES)
        build_program(nc)
        nc.compile()
        _CACHE["nc"] = nc
    return _CACHE["nc"]


def make_in_map(
    f_x, x_grid, q_weights, lift_W, lift_b, pw_W, pw_b, ker_log_ell, ker_log_sigma,
    proj1_W, proj1_b, proj2_W, proj2_b, proj3_W, proj3_b,
):
    f8 = lambda a: np.asarray(a, dtype=np.float64)
    x = f8(x_grid).reshape(N)
    w = f8(q_weights).reshape(N)
    f = f8(f_x).reshape(N)
    a = 0.5 * np.exp(-2.0 * f8(ker_log_ell))          # [DEPTH, C]
    sig2 = np.exp(2.0 * f8(ker_log_sigma))            # [DEPTH, C]
    ks = np.arange(K, dtype=np.float64)
    lnfact = np.concatenate([[0.0], np.cumsum(np.log(np.arange(1, K)))])

    p = np.arange(128)
    npr = np.arange(512)

    # host lift: fqT[32j+c, n'] = gelu([f,x] @ lift_W.T + lift_b)[512j+n', c]
    pre = np.stack([f, x], axis=1) @ f8(lift_W).T + f8(lift_b)  # [N, C]
    fql = 0.5 * pre * (1.0 + np.tanh(
        0.7978845608028654 * (pre + 0.044715 * pre ** 3)))
    fqT = np.zeros((128, 512), np.float64)
    for j in range(4):
        fqT[32 * j:32 * (j + 1), :] = fql[512 * j:512 * (j + 1), :].T

    def bd(W):  # block-diag lhsT: [32j+c', 32j+c] = W[c, c']
        M = np.zeros((128, 128), np.float64)
        for j in range(4):
            M[32 * j:32 * (j + 1), 32 * j:32 * (j + 1)] = f8(W).T
        return M

    def tmpN_of(i):
        t = np.zeros((128, 512), np.float64)
        for m in range(4):
            for j in range(4):
                q = 512 * j + 128 * m + p
                t[:, 128 * m + 32 * j:128 * m + 32 * j + 32] = (
                    w[q, None] * np.exp(-a[i][None, :] * (x[q, None] ** 2)))
        return t

    def EsT_of(i):
        e = np.zeros((128, 512), np.float64)
        for j in range(4):
            nn = 512 * j + npr
            e[32 * j:32 * (j + 1), :] = (
                sig2[i][:, None] * np.exp(-a[i][:, None] * (x[None, nn] ** 2)))
        return e

    def B4M_of(i):
        # eval stationary mask: rows 16g+k (8 replication groups); only
        # group g==j contributes to output node-chunk j, rest stay zero
        M = np.zeros((128, 128), np.float64)
        B = np.exp(ks[:, None] * np.log(2.0 * a[i][None, :]) - lnfact[:, None])
        for j in range(4):
            M[K * j:K * (j + 1), 32 * j:32 * (j + 1)] = B
        return M

    # moment basis Vm[p, K(4m+j)+k] = x_{512j+128m+p}^k
    dvm = np.zeros((128, 16 * K), np.float64)
    for m in range(4):
        for j in range(4):
            q = 512 * j + 128 * m + p
            dvm[:, K * (4 * m + j):K * (4 * m + j + 1)] = x[q, None] ** ks[None, :]

    tmpN0 = tmpN_of(0)
    da = np.concatenate(
        [fqT[:, 0:256], dvm[:, 0:8 * K], tmpN0[:, 0:256]], axis=1)
    db = np.concatenate(
        [fqT[:, 256:512], dvm[:, 8 * K:16 * K], tmpN0[:, 256:512]], axis=1)

    cfa = np.zeros((128, CF_W), np.float64)
    cfa[:, 0] = np.tile(f8(pw_b)[0], 4)
    cfa[:, 1] = np.tile(f8(pw_b)[1], 4)
    cfa[:, 2] = np.tile(f8(proj1_b) + f8(proj1_W) @ f8(pw_b)[2], 4)
    cfa[:, 3] = np.tile(f8(proj2_b), 4)
    cfa[0:4, 4] = f8(proj3_b)[0]
    cf16 = cfa.astype(np.float32).view(NPBF16)  # raw f32 halves as bf16 cols

    dgp = np.concatenate(
        [B4M_of(0).astype(NPBF16), bd(pw_W[0]).astype(NPBF16), cf16], axis=1)

    dy = np.zeros((128, 512), np.float64)
    for j in range(4):
        dy[K * j:K * (j + 1), 0:512] = (
            x[None, 512 * j:512 * (j + 1)] ** ks[:, None])

    dhd = np.zeros((128, DHD_W), np.float64)
    dhd[:, 0:128] = bd(proj1_W)
    dhd[:, 128:256] = bd(proj2_W)
    for j in range(4):
        dhd[32 * j:32 * (j + 1), 256 + j] = f8(proj3_W)[0]
    dhd[:, 260:388] = bd(f8(proj1_W) @ f8(pw_W[2]))

    def dl_of(i):
        dl = np.zeros((128, DL_W), np.float64)
        dl[:, 0:512] = tmpN_of(i)
        dl[:, 512:1024] = EsT_of(i)
        dl[:, 1024:1152] = B4M_of(i)
        dl[:, 1152:1280] = bd(pw_W[i])
        return dl

    return {
        "da": da.astype(NPBF16),
        "db": db.astype(NPBF16),
        "dg": dgp,
        "dy": dy.astype(NPBF16),
        "de0": EsT_of(0).astype(NPBF16),
        "dhd": dhd.astype(NPBF16),
        "dl1": dl_of(1).astype(NPBF16),
        "dl2": dl_of(2).astype(NPBF16),
    }


def kernel(**inputs) -> np.ndarray:
    nc = get_nc()
    in_map = make_in_map(**inputs)
    res = run_bass_kernel_spmd(nc, [in_map] * NCORES, list(range(NCORES)))
    return np.asarray(res.results[0]["out"], dtype=np.float32).reshape(N)
